# revision 43
# baseline (speedup 1.0000x reference)
"""Trainium2 Bass kernel for nn_Assistance (colors_only path).

For each of 64x64=4096 patches (21x21 window, stride 2) of a 147x147x3
image: compute 3 wedge indicators from 5 params (ests), then the
wedge-weighted mean colors -> output (1, 3, 3, 64, 64).

Sharding: 8 cores x 8 patch rows (512 patches each); each core gets its
35 relevant image rows + ests shard (SPMD graph is core-independent).

Key tricks:
  - num_k,c = (S_c - M1_c, M1_c - M2_c, M2_c), wsum = (441-H1, H1-H2, H2)
    with M1 = sum(h0*I), M2 = sum(h0*h1*I): only 6 fused mult+accum ops.
  - x[r,s]=grid[s], y[r,s]=grid[r]: per-angle affines run on [128,21]
    tiles (GpSimd) and combine via stride-0-broadcast reads in one
    441-wide DVE add per line.
  - wedge sign folded into per-patch line coefficients.
  - sin/cos via half-angle deg-7/8 polys (v = a/2 - pi/2 wrapped).
  - a4's mod-condition == (a1 < a3) exactly.
  - one stacked Arctan [d13|d42] on ACT; patch deinterleave on ACT.
"""
import os
import sys

for _p in ("/opt/trn_rl_repo", "/root/.axon_site/_ro/trn_rl_repo"):
    if os.path.isdir(_p) and _p not in sys.path:
        sys.path.insert(0, _p)

import numpy as np

import concourse.bass as bass
import concourse.bacc as bacc
import concourse.tile as tile
from concourse import mybir
from concourse.bass_utils import run_bass_kernel_spmd

F32 = mybir.dt.float32
OP = mybir.AluOpType
ACT = mybir.ActivationFunctionType

PI = float(np.pi)
R = 21
STRIDE = 2
ETA = 0.01
TAU = 0.1
H = W = 147
HP = WP = 64
NPIX = R * R
NCORES = 8
GROUPS = 4
ROWS_PER_CORE = 35


def _fit_half_angle_coeffs():
    v = np.linspace(-PI / 2, PI / 2, 20001)
    A = np.stack([v ** (2 * k + 1) for k in range(4)], 1)
    cs = np.linalg.lstsq(A, np.sin(v), rcond=None)[0]
    Ac = np.stack([v ** (2 * k) for k in range(5)], 1)
    cc = np.linalg.lstsq(Ac, -2 * np.cos(v), rcond=None)[0]
    return [float(x) for x in cs], [float(x) for x in cc]


SIN_C, COS_C = _fit_half_angle_coeffs()


def build_nc():
    nc = bacc.Bacc()

    img_ext = nc.declare_dram_parameter("img", [ROWS_PER_CORE * W * 3], F32, isOutput=False)
    ests_ext = nc.declare_dram_parameter("ests", [512 * 5], F32, isOutput=False)
    xg_ext = nc.declare_dram_parameter("xg", [NPIX], F32, isOutput=False)
    yg_ext = nc.declare_dram_parameter("yg", [NPIX], F32, isOutput=False)
    out_ext = nc.declare_dram_parameter("out", [128, 36], F32, isOutput=True)

    def bcast(ext, n):
        a = ext[:]
        return bass.AP(tensor=a.tensor, offset=a.offset, ap=[[0, 128], [1, n]])

    def dram_ap(ext, offset, dims):
        a = ext[:]
        return bass.AP(tensor=a.tensor, offset=a.offset + offset, ap=dims)

    with tile.TileContext(nc) as tc:
        with (
            tc.tile_pool(name="const", bufs=1) as const,
            tc.tile_pool(name="sc", bufs=1) as sc,
            tc.tile_pool(name="patch", bufs=1) as patchp,
            tc.tile_pool(name="work", bufs=4) as work,
        ):
            G = GROUPS
            V, GP = nc.vector, nc.gpsimd

            # ---------------- constants (E first; spread queues) ----------------
            E = const.tile([128, 5, GROUPS], F32)
            nc.sync.dma_start(
                out=E, in_=dram_ap(ests_ext, 0, [[5 * GROUPS, 128], [GROUPS, 5], [1, GROUPS]])
            )
            X4 = const.tile([128, NPIX], F32)
            nc.scalar.dma_start(out=X4, in_=bcast(xg_ext, NPIX))
            Y4 = const.tile([128, NPIX], F32)
            nc.scalar.dma_start(out=Y4, in_=bcast(yg_ext, NPIX))
            HALFCOL = const.tile([128, 1], F32)
            nc.vector.memset(HALFCOL, 0.5)

            def mk(pool, shape, tag):
                return pool.tile(shape, F32, name=tag, tag=tag)

            def ts(eng, out, in0, s1, s2=None, op0=OP.mult, op1=OP.add):
                if s2 is None:
                    eng.tensor_scalar(out=out, in0=in0, scalar1=s1, scalar2=None, op0=op0)
                else:
                    eng.tensor_scalar(out=out, in0=in0, scalar1=s1, scalar2=s2, op0=op0, op1=op1)

            def stt(out, in0, s, in1, op0, op1, accum_out=None):
                kw = {"accum_out": accum_out} if accum_out is not None else {}
                nc.vector.scalar_tensor_tensor(
                    out=out, in0=in0, scalar=s, in1=in1, op0=op0, op1=op1, **kw
                )

            def tt(eng, out, a, b, op):
                eng.tensor_tensor(out=out, in0=a, in1=b, op=op)

            def t4(tag):
                return mk(sc, [128, G], tag)

            def t16(tag):
                return mk(sc, [128, 4 * G], tag)

            # ------------- per-patch scalars -------------
            # thetas (GP): TH[p,(g,q)] = mod((e+1)pi, 2pi)
            E3 = E[:, 0:3, :].rearrange("p q g -> p g q")
            T12 = mk(sc, [128, G, 3], "T12")
            T12F = T12[:, :, :].rearrange("p g q -> p (g q)")
            ts(V, T12[:, :, :], E3, PI, PI)
            cacc = None
            for i, j in enumerate((-1.0, 0.0, 1.0, 2.0)):
                cj = mk(sc, [128, G * 3], f"cj{i % 2}")
                ts(V, cj, T12F, j * 2 * PI, None, OP.is_ge)
                if cacc is None:
                    cacc = cj
                else:
                    nxt = mk(sc, [128, G * 3], f"ca{i % 2}")
                    tt(V, nxt, cacc, cj, OP.add)
                    cacc = nxt
            mterm = mk(sc, [128, G * 3], "mterm")
            ts(V, mterm, cacc, -2 * PI, 4 * PI)
            TH = mk(sc, [128, G, 3], "TH")
            tt(V, TH[:, :, :].rearrange("p g q -> p (g q)"), T12F, mterm, OP.add)

            # sort via reduce (DVE): a1 = min_q, a3 = max_q, a2 = sum - a1 - a3
            a1 = t4("a1"); a3 = t4("a3"); a2 = t4("a2"); asum = t4("asum")
            V.tensor_reduce(out=a1, in_=TH[:, :, :], axis=mybir.AxisListType.X, op=OP.min)
            V.tensor_reduce(out=a3, in_=TH[:, :, :], axis=mybir.AxisListType.X, op=OP.max)
            V.tensor_reduce(out=asum, in_=TH[:, :, :], axis=mybir.AxisListType.X, op=OP.add)
            sm1 = t4("sm1")
            tt(V, sm1, asum, a1, OP.subtract)
            tt(V, a2, sm1, a3, OP.subtract)

            x0 = t4("x0"); y0 = t4("y0")
            ts(V, x0, E[:, 3, :], 3.0, None)
            ts(V, y0, E[:, 4, :], 3.0, None)

            # a4 = 0.5*(a1+a3) + pi*[a1 < a3]   (exact for distinct angles)
            ge = t4("ge"); gpi = t4("gpi"); a4h = t4("a4h"); a4 = t4("a4")
            tt(V, ge, a1, a3, OP.is_lt)
            ts(V, gpi, ge, PI, None)
            stt(a4h, a3, 0.5, gpi, OP.mult, OP.add)
            stt(a4, a1, 0.5, a4h, OP.mult, OP.add)

            # Dst[:,0,:]=D13, [:,1,:]=D42
            Dst = mk(sc, [128, 2, G], "Dst")
            DstF = Dst[:, :, :].rearrange("p a g -> p (a g)")
            tt(V, Dst[:, 0, :], a3, a1, OP.subtract)
            dd = t4("dd")
            tt(V, dd, a2, a4, OP.subtract)
            ca = t4("ca"); cb = t4("cb"); cs_ = t4("cs_"); mt = t4("mt")
            ts(V, ca, dd, -2 * PI, None, OP.is_ge)
            ts(V, cb, dd, 0.0, None, OP.is_ge)
            tt(V, cs_, ca, cb, OP.add)
            ts(V, mt, cs_, -2 * PI, 4 * PI)
            tt(V, Dst[:, 1, :], dd, mt, OP.add)

            # sgn stacked [128, 2, G] (dim1: 0=13, 1=42)
            SGN = mk(sc, [128, 2, G], "SGN")
            NSGN = mk(sc, [128, 2, G], "NSGN")
            clt = mk(sc, [128, 2 * G], "clt")
            ts(V, clt, DstF, PI, None, OP.is_lt)
            ts(V, SGN[:, :, :].rearrange("p a g -> p (a g)"), clt, 2.0, -1.0)
            ts(V, NSGN[:, :, :].rearrange("p a g -> p (a g)"),
               SGN[:, :, :].rearrange("p a g -> p (a g)"), -1.0, None)

            # gt stacked: tau*(D/pi - 1)^35 on [128, 2G] (GP)
            def t8(tag):
                return mk(sc, [128, 2 * G], tag)
            gv = t8("gv")
            ts(V, gv, DstF, 1.0 / PI, -1.0)
            gv2 = t8("gv2"); tt(V, gv2, gv, gv, OP.mult)
            gv3 = t8("gv3"); tt(V, gv3, gv2, gv, OP.mult)
            gv4 = t8("gv4"); tt(V, gv4, gv2, gv2, OP.mult)
            gv8 = t8("gv8"); tt(V, gv8, gv4, gv4, OP.mult)
            gv16 = t8("gv16"); tt(V, gv16, gv8, gv8, OP.mult)
            gv32 = t8("gv32"); tt(V, gv32, gv16, gv16, OP.mult)
            gv35 = t8("gv35"); tt(V, gv35, gv32, gv3, OP.mult)
            GTS = mk(sc, [128, 2, G], "GTS")
            ts(V, GTS[:, :, :].rearrange("p a g -> p (a g)"), gv35, TAU, None)

            # ---- sin/cos via half-angle, ROLE order (a4, a2, a1, a3) ----
            v0t = mk(sc, [128, 4, G], "v0t")
            for i, a in enumerate((a4, a2, a1, a3)):
                ts(V, v0t[:, i, :], a, 0.5, -PI / 2)
            v0 = v0t[:, :, :].rearrange("p a g -> p (a g)")
            cwr = t16("cwr")
            ts(V, cwr, v0, PI / 2, None, OP.is_ge)
            vv = t16("vv")
            stt(vv, cwr, -PI, v0, OP.mult, OP.add)
            v2t = t16("v2t")
            tt(V, v2t, vv, vv, OP.mult)
            sp = t16("spa")
            ts(V, sp, v2t, SIN_C[3], SIN_C[2])
            for k in (1, 0):
                q = t16(f"spq{k}")
                tt(V, q, sp, v2t, OP.mult)
                sp2 = t16(f"sp{k}")
                ts(V, sp2, q, SIN_C[k], None, OP.add)
                sp = sp2
            SV = t16("SV")
            tt(V, SV, sp, vv, OP.mult)
            cp = t16("cpa")
            ts(V, cp, v2t, COS_C[4], COS_C[3])
            for k in (2, 1, 0):
                qc = t16(f"cpq{k}")
                tt(V, qc, cp, v2t, OP.mult)
                cp2 = t16(f"cp{k}")
                ts(V, cp2, qc, COS_C[k], None, OP.add)
                cp = cp2
            SIN = mk(sc, [128, 4, G], "SIN")  # role-ordered sin(angle)
            tt(V, SIN[:, :, :].rearrange("p a g -> p (a g)"), SV, cp, OP.mult)
            sqv = t16("sqv")
            tt(V, sqv, SV, SV, OP.mult)
            COS = mk(sc, [128, 4, G], "COS")
            ts(V, COS[:, :, :].rearrange("p a g -> p (a g)"), sqv, 2.0, -1.0)

            # ---- role-folded coefficients [128, 4, G] ----
            # role multipliers s_r = (sgn42, -sgn42, sgn13, -sgn13)
            # SGS = -s_r = (nsg42, sgn42, nsg13, sgn13)
            SGS = mk(sc, [128, 4, G], "SGS")
            V.tensor_copy(out=SGS[:, 0, :], in_=NSGN[:, 1, :])
            V.tensor_copy(out=SGS[:, 1, :], in_=SGN[:, 1, :])
            V.tensor_copy(out=SGS[:, 2, :], in_=NSGN[:, 0, :])
            V.tensor_copy(out=SGS[:, 3, :], in_=SGN[:, 0, :])
            PSG = mk(sc, [128, 4, G], "PSG")  # +s_r
            ts(V, PSG[:, :, :].rearrange("p a g -> p (a g)"),
               SGS[:, :, :].rearrange("p a g -> p (a g)"), -1.0, None)

            NSG = mk(sc, [128, 4, G], "NSG")  # -sin*s = sin*SGS
            CG = mk(sc, [128, 4, G], "CG")    # cos*s
            tt(V, NSG[:, :, :].rearrange("p a g -> p (a g)"),
               SIN[:, :, :].rearrange("p a g -> p (a g)"),
               SGS[:, :, :].rearrange("p a g -> p (a g)"), OP.mult)
            tt(V, CG[:, :, :].rearrange("p a g -> p (a g)"),
               COS[:, :, :].rearrange("p a g -> p (a g)"),
               PSG[:, :, :].rearrange("p a g -> p (a g)"), OP.mult)
            # DG = -(NSG*x0 + CG*y0)  (x0, y0 broadcast over role dim)
            def rep4(t):
                return bass.AP(tensor=t.tensor, offset=t.offset,
                               ap=[t.ap[0], [0, 4], [1, G]])
            u1 = mk(sc, [128, 4, G], "u1")
            tt(V, u1[:, :, :], NSG[:, :, :], rep4(x0), OP.mult)
            u2 = mk(sc, [128, 4, G], "u2")
            tt(V, u2[:, :, :], CG[:, :, :], rep4(y0), OP.mult)
            u3 = mk(sc, [128, 4, G], "u3")
            tt(V, u3[:, :, :], u1[:, :, :], u2[:, :, :], OP.add)
            DG = mk(sc, [128, 4, G], "DG")
            ts(V, DG[:, :, :].rearrange("p a g -> p (a g)"),
               u3[:, :, :].rearrange("p a g -> p (a g)"), -1.0, None)

            # scaled copies for folding the distance affine into Arctan
            SGN100 = mk(sc, [128, 2, G], "SGN100")
            ts(V, SGN100[:, :, :].rearrange("p a g -> p (a g)"),
               SGN[:, :, :].rearrange("p a g -> p (a g)"), 1.0 / ETA, None)
            GTS100 = mk(sc, [128, 2, G], "GTS100")
            ts(V, GTS100[:, :, :].rearrange("p a g -> p (a g)"),
               GTS[:, :, :].rearrange("p a g -> p (a g)"), 1.0 / ETA, None)

            # ---- accumulators (split per writing engine to avoid false deps) ----
            RM = const.tile([128, 32], F32)   # DVE: M1 @ c*4+g, M2 @ 16+c*4+g
            RS = const.tile([128, 12], F32)   # ACT: S @ c*4+g
            RH1 = const.tile([128, 4], F32)   # ACT: H1 @ g
            RH2 = const.tile([128, 4], F32)   # DVE: H2 @ g

            # ---------------- patch prefetch + S_c accums ----------------
            patches = []
            dma_engs = [nc.sync, nc.scalar, nc.sync, nc.scalar]
            for g in range(GROUPS):
                patch = mk(patchp, [128, R, 63], f"patch{g}")
                for dh in range(2):
                    row0 = 4 * g + 2 * dh
                    dma_engs[g].dma_start(
                        out=patch[dh * 64:(dh + 1) * 64, :, :],
                        in_=dram_ap(img_ext, row0 * W * 3,
                                    [[STRIDE * 3, 64], [W * 3, R], [1, 63]]),
                    )
                patches.append(patch)
            for g in range(GROUPS):
                for c in range(3):
                    so = mk(work, [128, R, R], "so")
                    nc.scalar.activation(out=so, in_=patches[g][:, :, c::3], func=ACT.Copy,
                                         accum_out=RS[:, c * 4 + g:c * 4 + g + 1])

            # ---------------- main loop (software-pipelined emission) ----------------
            fronts = {}

            def emit_front(g):
                pv = patches[g][:, :, :]
                Lu = mk(work, [128, 2, NPIX], "Lu")
                Lv = mk(work, [128, 2, NPIX], "Lv")
                for r, (dst, half) in enumerate(((Lu, 0), (Lv, 0), (Lu, 1), (Lv, 1))):
                    px = mk(work, [128, NPIX], f"px{r}")
                    if r % 2 == 0:
                        nc.scalar.activation(out=px, in_=X4, func=ACT.Identity,
                                             scale=NSG[:, r, g:g + 1], bias=DG[:, r, g:g + 1])
                    else:
                        V.tensor_scalar(out=px, in0=X4, scalar1=NSG[:, r, g:g + 1],
                                        scalar2=DG[:, r, g:g + 1], op0=OP.mult, op1=OP.add)
                    stt(dst[:, half, :], Y4, CG[:, r, g:g + 1], px, OP.mult, OP.add)
                MN = mk(work, [128, 2, NPIX], "MN")
                tt(V, MN[:, :, :], Lu[:, :, :], Lv[:, :, :], OP.min)
                T = mk(work, [128, 2 * NPIX], "T")
                nc.scalar.activation(out=T[:, 0:NPIX], in_=MN[:, 1, :], func=ACT.Arctan,
                                     scale=SGN100[:, 0, g:g + 1], bias=GTS100[:, 0, g:g + 1])
                nc.scalar.activation(out=T[:, NPIX:], in_=MN[:, 0, :], func=ACT.Arctan,
                                     scale=SGN100[:, 1, g:g + 1], bias=GTS100[:, 1, g:g + 1])
                h0 = mk(work, [128, NPIX], "h0")
                nc.scalar.activation(out=h0, in_=T[:, 0:NPIX], func=ACT.Identity,
                                     scale=1.0 / PI, bias=HALFCOL,
                                     accum_out=RH1[:, g:g + 1])
                h1 = mk(work, [128, NPIX], "h1")
                nc.scalar.activation(out=h1, in_=T[:, NPIX:], func=ACT.Identity,
                                     scale=1.0 / PI, bias=HALFCOL)
                fronts[g] = (h0, h1)

            def emit_back(g):
                pv = patches[g][:, :, :]
                h0, h1 = fronts[g]
                p01 = mk(work, [128, NPIX], "p01")
                stt(p01, h0, 1.0, h1, OP.mult, OP.mult,
                    accum_out=RH2[:, g:g + 1])
                h0v = h0[:, :].rearrange("p (r s) -> p r s", r=R)
                p01v = p01[:, :].rearrange("p (r s) -> p r s", r=R)
                for c in range(3):
                    col = c * 4 + g
                    Ic = pv[:, :, c::3]
                    m1o = mk(work, [128, R, R], "m1o")
                    stt(m1o, Ic, 1.0, h0v, OP.mult, OP.mult,
                        accum_out=RM[:, col:col + 1])
                    m2o = mk(work, [128, R, R], "m2o")
                    stt(m2o, Ic, 1.0, p01v, OP.mult, OP.mult,
                        accum_out=RM[:, col + 16:col + 17])

            emit_front(0)
            emit_front(1)
            emit_back(0)
            emit_front(2)
            emit_back(1)
            emit_front(3)
            emit_back(2)
            emit_back(3)

            # ---------------- epilogue ----------------
            Sv = RS[:, 0:12].rearrange("p (c g) -> p c g", c=3)
            M1v = RM[:, 0:12].rearrange("p (c g) -> p c g", c=3)
            M2v = RM[:, 16:28].rearrange("p (c g) -> p c g", c=3)
            H1v = RH1[:, 0:4]
            H2v = RH2[:, 0:4]

            C = const.tile([128, 3, 3, GROUPS], F32)  # (c, k, g)
            tt(V, C[:, :, 0, :], Sv, M1v, OP.subtract)
            tt(V, C[:, :, 1, :], M1v, M2v, OP.subtract)
            V.tensor_copy(out=C[:, :, 2, :], in_=M2v)

            Wt = const.tile([128, 3, GROUPS], F32)  # (k, g)
            ts(V, Wt[:, 0, :], H1v, -1.0, float(NPIX))
            tt(V, Wt[:, 1, :], H1v, H2v, OP.subtract)
            V.tensor_copy(out=Wt[:, 2, :], in_=H2v)

            W2 = const.tile([128, 3 * GROUPS], F32)
            ts(V, W2, Wt[:, :, :].rearrange("p k g -> p (k g)"), 1e-10, None, OP.add)
            VW = const.tile([128, 3, GROUPS], F32)
            nc.vector.reciprocal(out=VW[:, :, :].rearrange("p k g -> p (k g)"), in_=W2)

            C2 = const.tile([128, 3, 3, GROUPS], F32)
            for c in range(3):
                tt(V, C2[:, c, :, :], C[:, c, :, :], VW[:, :, :], OP.mult)

            nc.sync.dma_start(
                out=out_ext[:, :],
                in_=C2[:, :, :, :].rearrange("p a b c -> p (a b c)"),
            )

    nc.finalize()
    return nc


_NC_CACHE = None


def _get_nc():
    global _NC_CACHE
    if _NC_CACHE is None:
        _NC_CACHE = build_nc()
    return _NC_CACHE


def make_in_maps(ests, noisy_image):
    img = np.ascontiguousarray(np.asarray(noisy_image, dtype=np.float32)[0])
    ests = np.asarray(ests, dtype=np.float32).reshape(HP * WP, 5)
    grid = np.linspace(-1.0, 1.0, R, dtype=np.float32)
    xg = np.tile(grid, R)
    yg = np.repeat(grid, R)
    in_maps = []
    for m in range(NCORES):
        in_maps.append({
            "img": np.ascontiguousarray(img[16 * m:16 * m + ROWS_PER_CORE]).reshape(-1),
            "ests": np.ascontiguousarray(
                ests[m * 512:(m + 1) * 512].reshape(GROUPS, 128, 5).transpose(1, 2, 0)).reshape(-1),
            "xg": xg, "yg": yg,
        })
    return in_maps


def assemble(results):
    out = np.empty((1, 3, 3, HP, WP), np.float32)
    for m in range(NCORES):
        r = results[m]["out"].reshape(2, 64, 3, 3, GROUPS)   # (dh, wp, c, k, g)
        out[0, :, :, 8 * m:8 * m + 8, :] = (
            r.transpose(2, 3, 4, 0, 1).reshape(3, 3, 8, WP))
    return out


def kernel(ests, noisy_image, gt_image=None, alpha=None, **_):
    nc = _get_nc()
    in_maps = make_in_maps(ests, noisy_image)
    res = run_bass_kernel_spmd(nc, in_maps, core_ids=list(range(NCORES)))
    return assemble(res.results)


# revision 44
# speedup vs baseline: 1.1480x; 1.1480x over previous
"""Trainium2 Bass kernel for nn_Assistance (colors_only path).

For each of 64x64=4096 patches (21x21 window, stride 2) of a 147x147x3
image: compute 3 wedge indicators from 5 params (ests), then the
wedge-weighted mean colors -> output (1, 3, 3, 64, 64).

Sharding: 8 cores x 8 patch rows (512 patches each); each core gets its
35 relevant image rows + ests shard (SPMD graph is core-independent).

Key tricks:
  - num_k,c = (S_c - M1_c, M1_c - M2_c, M2_c), wsum = (441-H1, H1-H2, H2)
    with M1 = sum(h0*I), M2 = sum(h0*h1*I): only 6 fused mult+accum ops.
  - x[r,s]=grid[s], y[r,s]=grid[r]: per-angle affines run on [128,21]
    tiles (GpSimd) and combine via stride-0-broadcast reads in one
    441-wide DVE add per line.
  - wedge sign folded into per-patch line coefficients.
  - sin/cos via half-angle deg-7/8 polys (v = a/2 - pi/2 wrapped).
  - a4's mod-condition == (a1 < a3) exactly.
  - one stacked Arctan [d13|d42] on ACT; patch deinterleave on ACT.
"""
import os
import sys

for _p in ("/opt/trn_rl_repo", "/root/.axon_site/_ro/trn_rl_repo"):
    if os.path.isdir(_p) and _p not in sys.path:
        sys.path.insert(0, _p)

import numpy as np

import concourse.bass as bass
import concourse.bacc as bacc
import concourse.tile as tile
from concourse import mybir
from concourse.bass_utils import run_bass_kernel_spmd

F32 = mybir.dt.float32
OP = mybir.AluOpType
ACT = mybir.ActivationFunctionType

PI = float(np.pi)
R = 21
STRIDE = 2
ETA = 0.01
TAU = 0.1
H = W = 147
HP = WP = 64
NPIX = R * R
NCORES = 8
GROUPS = 4
ROWS_PER_CORE = 35


def _fit_half_angle_coeffs():
    v = np.linspace(-PI / 2, PI / 2, 20001)
    A = np.stack([v ** (2 * k + 1) for k in range(4)], 1)
    cs = np.linalg.lstsq(A, np.sin(v), rcond=None)[0]
    Ac = np.stack([v ** (2 * k) for k in range(5)], 1)
    cc = np.linalg.lstsq(Ac, -2 * np.cos(v), rcond=None)[0]
    return [float(x) for x in cs], [float(x) for x in cc]


SIN_C, COS_C = _fit_half_angle_coeffs()


def build_nc():
    nc = bacc.Bacc()

    img_ext = nc.declare_dram_parameter("img", [ROWS_PER_CORE * W * 3], F32, isOutput=False)
    ests_ext = nc.declare_dram_parameter("ests", [512 * 5], F32, isOutput=False)
    xg_ext = nc.declare_dram_parameter("xg", [NPIX], F32, isOutput=False)
    yg_ext = nc.declare_dram_parameter("yg", [NPIX], F32, isOutput=False)
    out_ext = nc.declare_dram_parameter("out", [128, 36], F32, isOutput=True)

    def bcast(ext, n):
        a = ext[:]
        return bass.AP(tensor=a.tensor, offset=a.offset, ap=[[0, 128], [1, n]])

    def dram_ap(ext, offset, dims):
        a = ext[:]
        return bass.AP(tensor=a.tensor, offset=a.offset + offset, ap=dims)

    with tile.TileContext(nc) as tc:
        with (
            tc.tile_pool(name="const", bufs=1) as const,
            tc.tile_pool(name="sc", bufs=1) as sc,
            tc.tile_pool(name="patch", bufs=1) as patchp,
            tc.tile_pool(name="work", bufs=4) as work,
        ):
            G = GROUPS
            V, GP = nc.vector, nc.gpsimd

            # ---------------- constants (E first; spread queues) ----------------
            E = const.tile([128, 5, GROUPS], F32)
            nc.sync.dma_start(
                out=E, in_=dram_ap(ests_ext, 0, [[5 * GROUPS, 128], [GROUPS, 5], [1, GROUPS]])
            )
            X4 = const.tile([128, NPIX], F32)
            nc.scalar.dma_start(out=X4, in_=bcast(xg_ext, NPIX))
            Y4 = const.tile([128, NPIX], F32)
            nc.scalar.dma_start(out=Y4, in_=bcast(yg_ext, NPIX))
            HALFCOL = const.tile([128, 1], F32)
            nc.vector.memset(HALFCOL, 0.5)

            def mk(pool, shape, tag):
                return pool.tile(shape, F32, name=tag, tag=tag)

            def ts(eng, out, in0, s1, s2=None, op0=OP.mult, op1=OP.add):
                if s2 is None:
                    eng.tensor_scalar(out=out, in0=in0, scalar1=s1, scalar2=None, op0=op0)
                else:
                    eng.tensor_scalar(out=out, in0=in0, scalar1=s1, scalar2=s2, op0=op0, op1=op1)

            def stt(out, in0, s, in1, op0, op1, accum_out=None):
                kw = {"accum_out": accum_out} if accum_out is not None else {}
                nc.vector.scalar_tensor_tensor(
                    out=out, in0=in0, scalar=s, in1=in1, op0=op0, op1=op1, **kw
                )

            def tt(eng, out, a, b, op):
                eng.tensor_tensor(out=out, in0=a, in1=b, op=op)

            def t4(tag):
                return mk(sc, [128, G], tag)

            def t16(tag):
                return mk(sc, [128, 4 * G], tag)

            # ------------- per-patch scalars -------------
            # thetas (GP): TH[p,(g,q)] = mod((e+1)pi, 2pi)
            E3 = E[:, 0:3, :].rearrange("p q g -> p g q")
            T12 = mk(sc, [128, G, 3], "T12")
            T12F = T12[:, :, :].rearrange("p g q -> p (g q)")
            ts(V, T12[:, :, :], E3, PI, PI)
            cacc = None
            for i, j in enumerate((-1.0, 0.0, 1.0, 2.0)):
                cj = mk(sc, [128, G * 3], f"cj{i % 2}")
                ts(V, cj, T12F, j * 2 * PI, None, OP.is_ge)
                if cacc is None:
                    cacc = cj
                else:
                    nxt = mk(sc, [128, G * 3], f"ca{i % 2}")
                    tt(V, nxt, cacc, cj, OP.add)
                    cacc = nxt
            mterm = mk(sc, [128, G * 3], "mterm")
            ts(V, mterm, cacc, -2 * PI, 4 * PI)
            TH = mk(sc, [128, G, 3], "TH")
            tt(V, TH[:, :, :].rearrange("p g q -> p (g q)"), T12F, mterm, OP.add)

            # sort via reduce (DVE): a1 = min_q, a3 = max_q, a2 = sum - a1 - a3
            a1 = t4("a1"); a3 = t4("a3"); a2 = t4("a2"); asum = t4("asum")
            V.tensor_reduce(out=a1, in_=TH[:, :, :], axis=mybir.AxisListType.X, op=OP.min)
            V.tensor_reduce(out=a3, in_=TH[:, :, :], axis=mybir.AxisListType.X, op=OP.max)
            V.tensor_reduce(out=asum, in_=TH[:, :, :], axis=mybir.AxisListType.X, op=OP.add)
            sm1 = t4("sm1")
            tt(V, sm1, asum, a1, OP.subtract)
            tt(V, a2, sm1, a3, OP.subtract)

            x0 = t4("x0"); y0 = t4("y0")
            ts(V, x0, E[:, 3, :], 3.0, None)
            ts(V, y0, E[:, 4, :], 3.0, None)

            # a4 = 0.5*(a1+a3) + pi*[a1 < a3]   (exact for distinct angles)
            ge = t4("ge"); gpi = t4("gpi"); a4h = t4("a4h"); a4 = t4("a4")
            tt(V, ge, a1, a3, OP.is_lt)
            ts(V, gpi, ge, PI, None)
            stt(a4h, a3, 0.5, gpi, OP.mult, OP.add)
            stt(a4, a1, 0.5, a4h, OP.mult, OP.add)

            # Dst[:,0,:]=D13, [:,1,:]=D42
            Dst = mk(sc, [128, 2, G], "Dst")
            DstF = Dst[:, :, :].rearrange("p a g -> p (a g)")
            tt(V, Dst[:, 0, :], a3, a1, OP.subtract)
            dd = t4("dd")
            tt(V, dd, a2, a4, OP.subtract)
            ca = t4("ca"); cb = t4("cb"); cs_ = t4("cs_"); mt = t4("mt")
            ts(V, ca, dd, -2 * PI, None, OP.is_ge)
            ts(V, cb, dd, 0.0, None, OP.is_ge)
            tt(V, cs_, ca, cb, OP.add)
            ts(V, mt, cs_, -2 * PI, 4 * PI)
            tt(V, Dst[:, 1, :], dd, mt, OP.add)

            # sgn stacked [128, 2, G] (dim1: 0=13, 1=42)
            SGN = mk(sc, [128, 2, G], "SGN")
            NSGN = mk(sc, [128, 2, G], "NSGN")
            clt = mk(sc, [128, 2 * G], "clt")
            ts(V, clt, DstF, PI, None, OP.is_lt)
            ts(V, SGN[:, :, :].rearrange("p a g -> p (a g)"), clt, 2.0, -1.0)
            ts(V, NSGN[:, :, :].rearrange("p a g -> p (a g)"),
               SGN[:, :, :].rearrange("p a g -> p (a g)"), -1.0, None)

            # gt stacked: tau*(D/pi - 1)^35 on [128, 2G] (GP)
            def t8(tag):
                return mk(sc, [128, 2 * G], tag)
            gv = t8("gv")
            ts(V, gv, DstF, 1.0 / PI, -1.0)
            gv2 = t8("gv2"); tt(V, gv2, gv, gv, OP.mult)
            gv3 = t8("gv3"); tt(V, gv3, gv2, gv, OP.mult)
            gv4 = t8("gv4"); tt(V, gv4, gv2, gv2, OP.mult)
            gv8 = t8("gv8"); tt(V, gv8, gv4, gv4, OP.mult)
            gv16 = t8("gv16"); tt(V, gv16, gv8, gv8, OP.mult)
            gv32 = t8("gv32"); tt(V, gv32, gv16, gv16, OP.mult)
            gv35 = t8("gv35"); tt(V, gv35, gv32, gv3, OP.mult)
            GTS = mk(sc, [128, 2, G], "GTS")
            ts(V, GTS[:, :, :].rearrange("p a g -> p (a g)"), gv35, TAU, None)

            # ---- sin/cos via half-angle, ROLE order (a4, a2, a1, a3) ----
            v0t = mk(sc, [128, 4, G], "v0t")
            for i, a in enumerate((a4, a2, a1, a3)):
                ts(V, v0t[:, i, :], a, 0.5, -PI / 2)
            v0 = v0t[:, :, :].rearrange("p a g -> p (a g)")
            cwr = t16("cwr")
            ts(V, cwr, v0, PI / 2, None, OP.is_ge)
            vv = t16("vv")
            stt(vv, cwr, -PI, v0, OP.mult, OP.add)
            v2t = t16("v2t")
            tt(V, v2t, vv, vv, OP.mult)
            sp = t16("spa")
            ts(V, sp, v2t, SIN_C[3], SIN_C[2])
            for k in (1, 0):
                q = t16(f"spq{k}")
                tt(V, q, sp, v2t, OP.mult)
                sp2 = t16(f"sp{k}")
                ts(V, sp2, q, SIN_C[k], None, OP.add)
                sp = sp2
            SV = t16("SV")
            tt(V, SV, sp, vv, OP.mult)
            cp = t16("cpa")
            ts(V, cp, v2t, COS_C[4], COS_C[3])
            for k in (2, 1, 0):
                qc = t16(f"cpq{k}")
                tt(V, qc, cp, v2t, OP.mult)
                cp2 = t16(f"cp{k}")
                ts(V, cp2, qc, COS_C[k], None, OP.add)
                cp = cp2
            SIN = mk(sc, [128, 4, G], "SIN")  # role-ordered sin(angle)
            tt(V, SIN[:, :, :].rearrange("p a g -> p (a g)"), SV, cp, OP.mult)
            sqv = t16("sqv")
            tt(V, sqv, SV, SV, OP.mult)
            COS = mk(sc, [128, 4, G], "COS")
            ts(V, COS[:, :, :].rearrange("p a g -> p (a g)"), sqv, 2.0, -1.0)

            # ---- role-folded coefficients [128, 4, G] ----
            # role multipliers s_r = (sgn42, -sgn42, sgn13, -sgn13)
            # SGS = -s_r = (nsg42, sgn42, nsg13, sgn13)
            SGS = mk(sc, [128, 4, G], "SGS")
            V.tensor_copy(out=SGS[:, 0, :], in_=NSGN[:, 1, :])
            V.tensor_copy(out=SGS[:, 1, :], in_=SGN[:, 1, :])
            V.tensor_copy(out=SGS[:, 2, :], in_=NSGN[:, 0, :])
            V.tensor_copy(out=SGS[:, 3, :], in_=SGN[:, 0, :])
            PSG = mk(sc, [128, 4, G], "PSG")  # +s_r
            ts(V, PSG[:, :, :].rearrange("p a g -> p (a g)"),
               SGS[:, :, :].rearrange("p a g -> p (a g)"), -1.0, None)

            NSG = mk(sc, [128, 4, G], "NSG")  # -sin*s = sin*SGS
            CG = mk(sc, [128, 4, G], "CG")    # cos*s
            tt(V, NSG[:, :, :].rearrange("p a g -> p (a g)"),
               SIN[:, :, :].rearrange("p a g -> p (a g)"),
               SGS[:, :, :].rearrange("p a g -> p (a g)"), OP.mult)
            tt(V, CG[:, :, :].rearrange("p a g -> p (a g)"),
               COS[:, :, :].rearrange("p a g -> p (a g)"),
               PSG[:, :, :].rearrange("p a g -> p (a g)"), OP.mult)
            # DG = -(NSG*x0 + CG*y0)  (x0, y0 broadcast over role dim)
            def rep4(t):
                return bass.AP(tensor=t.tensor, offset=t.offset,
                               ap=[t.ap[0], [0, 4], [1, G]])
            u1 = mk(sc, [128, 4, G], "u1")
            tt(V, u1[:, :, :], NSG[:, :, :], rep4(x0), OP.mult)
            u2 = mk(sc, [128, 4, G], "u2")
            tt(V, u2[:, :, :], CG[:, :, :], rep4(y0), OP.mult)
            u3 = mk(sc, [128, 4, G], "u3")
            tt(V, u3[:, :, :], u1[:, :, :], u2[:, :, :], OP.add)
            DG = mk(sc, [128, 4, G], "DG")
            ts(V, DG[:, :, :].rearrange("p a g -> p (a g)"),
               u3[:, :, :].rearrange("p a g -> p (a g)"), -1.0, None)

            # scaled copies for folding the distance affine into Arctan
            SGN100 = mk(sc, [128, 2, G], "SGN100")
            ts(V, SGN100[:, :, :].rearrange("p a g -> p (a g)"),
               SGN[:, :, :].rearrange("p a g -> p (a g)"), 1.0 / ETA, None)
            GTS100 = mk(sc, [128, 2, G], "GTS100")
            ts(V, GTS100[:, :, :].rearrange("p a g -> p (a g)"),
               GTS[:, :, :].rearrange("p a g -> p (a g)"), 1.0 / ETA, None)

            # ---- accumulators (split per writing engine to avoid false deps) ----
            RM = const.tile([128, 32], F32)   # DVE: M1 @ c*4+g, M2 @ 16+c*4+g
            RS = const.tile([128, 12], F32)   # ACT: S @ c*4+g
            RH1 = const.tile([128, 4], F32)   # ACT: H1 @ g
            RH2 = const.tile([128, 4], F32)   # DVE: H2 @ g

            # ---------------- patch prefetch + S_c accums ----------------
            patches = []
            dma_engs = [nc.sync, nc.scalar, nc.sync, nc.scalar]
            for g in range(GROUPS):
                patch = mk(patchp, [128, R, 63], f"patch{g}")
                for dh in range(2):
                    row0 = 4 * g + 2 * dh
                    dma_engs[g].dma_start(
                        out=patch[dh * 64:(dh + 1) * 64, :, :],
                        in_=dram_ap(img_ext, row0 * W * 3,
                                    [[STRIDE * 3, 64], [W * 3, R], [1, 63]]),
                    )
                patches.append(patch)
            for g in range(GROUPS):
                for c in range(3):
                    so = mk(work, [128, R, R], "so")
                    nc.scalar.activation(out=so, in_=patches[g][:, :, c::3], func=ACT.Copy,
                                         accum_out=RS[:, c * 4 + g:c * 4 + g + 1])

            # ---------------- main loop (software-pipelined emission) ----------------
            fronts = {}

            def emit_front(g):
                pv = patches[g][:, :, :]
                Lu = mk(work, [128, 2, NPIX], "Lu")
                Lv = mk(work, [128, 2, NPIX], "Lv")
                for r, (dst, half) in enumerate(((Lu, 0), (Lv, 0), (Lu, 1), (Lv, 1))):
                    px = mk(work, [128, NPIX], f"px{r}")
                    if r % 2 == 0:
                        nc.scalar.activation(out=px, in_=X4, func=ACT.Identity,
                                             scale=NSG[:, r, g:g + 1], bias=DG[:, r, g:g + 1])
                    else:
                        V.tensor_scalar(out=px, in0=X4, scalar1=NSG[:, r, g:g + 1],
                                        scalar2=DG[:, r, g:g + 1], op0=OP.mult, op1=OP.add)
                    stt(dst[:, half, :], Y4, CG[:, r, g:g + 1], px, OP.mult, OP.add)
                MN = mk(work, [128, 2, NPIX], "MN")
                tt(V, MN[:, :, :], Lu[:, :, :], Lv[:, :, :], OP.min)
                T = mk(work, [128, 2 * NPIX], "T")
                nc.scalar.activation(out=T[:, 0:NPIX], in_=MN[:, 1, :], func=ACT.Arctan,
                                     scale=SGN100[:, 0, g:g + 1], bias=GTS100[:, 0, g:g + 1])
                nc.scalar.activation(out=T[:, NPIX:], in_=MN[:, 0, :], func=ACT.Arctan,
                                     scale=SGN100[:, 1, g:g + 1], bias=GTS100[:, 1, g:g + 1])
                h0 = mk(work, [128, NPIX], "h0")
                nc.scalar.activation(out=h0, in_=T[:, 0:NPIX], func=ACT.Identity,
                                     scale=1.0 / PI, bias=HALFCOL,
                                     accum_out=RH1[:, g:g + 1])
                h1 = mk(work, [128, NPIX], "h1")
                nc.scalar.activation(out=h1, in_=T[:, NPIX:], func=ACT.Identity,
                                     scale=1.0 / PI, bias=HALFCOL)
                fronts[g] = (h0, h1)

            def emit_back(g):
                pv = patches[g][:, :, :]
                h0, h1 = fronts[g]
                p01 = mk(work, [128, NPIX], "p01")
                stt(p01, h0, 1.0, h1, OP.mult, OP.mult,
                    accum_out=RH2[:, g:g + 1])
                h0v = h0[:, :].rearrange("p (r s) -> p r s", r=R)
                p01v = p01[:, :].rearrange("p (r s) -> p r s", r=R)
                for c in range(3):
                    col = c * 4 + g
                    Ic = pv[:, :, c::3]
                    m1o = mk(work, [128, R, R], "m1o")
                    stt(m1o, h0v, 1.0, Ic, OP.mult, OP.mult,
                        accum_out=RM[:, col:col + 1])
                    m2o = mk(work, [128, R, R], "m2o")
                    stt(m2o, p01v, 1.0, Ic, OP.mult, OP.mult,
                        accum_out=RM[:, col + 16:col + 17])

            emit_front(0)
            emit_front(1)
            emit_back(0)
            emit_front(2)
            emit_back(1)
            emit_front(3)
            emit_back(2)
            emit_back(3)

            # ---------------- epilogue ----------------
            Sv = RS[:, 0:12].rearrange("p (c g) -> p c g", c=3)
            M1v = RM[:, 0:12].rearrange("p (c g) -> p c g", c=3)
            M2v = RM[:, 16:28].rearrange("p (c g) -> p c g", c=3)
            H1v = RH1[:, 0:4]
            H2v = RH2[:, 0:4]

            C = const.tile([128, 3, 3, GROUPS], F32)  # (c, k, g)
            tt(V, C[:, :, 0, :], Sv, M1v, OP.subtract)
            tt(V, C[:, :, 1, :], M1v, M2v, OP.subtract)
            V.tensor_copy(out=C[:, :, 2, :], in_=M2v)

            Wt = const.tile([128, 3, GROUPS], F32)  # (k, g)
            ts(V, Wt[:, 0, :], H1v, -1.0, float(NPIX))
            tt(V, Wt[:, 1, :], H1v, H2v, OP.subtract)
            V.tensor_copy(out=Wt[:, 2, :], in_=H2v)

            W2 = const.tile([128, 3 * GROUPS], F32)
            ts(V, W2, Wt[:, :, :].rearrange("p k g -> p (k g)"), 1e-10, None, OP.add)
            VW = const.tile([128, 3, GROUPS], F32)
            nc.vector.reciprocal(out=VW[:, :, :].rearrange("p k g -> p (k g)"), in_=W2)

            C2 = const.tile([128, 3, 3, GROUPS], F32)
            for c in range(3):
                tt(V, C2[:, c, :, :], C[:, c, :, :], VW[:, :, :], OP.mult)

            nc.sync.dma_start(
                out=out_ext[:, :],
                in_=C2[:, :, :, :].rearrange("p a b c -> p (a b c)"),
            )

    nc.finalize()
    return nc


_NC_CACHE = None


def _get_nc():
    global _NC_CACHE
    if _NC_CACHE is None:
        _NC_CACHE = build_nc()
    return _NC_CACHE


def make_in_maps(ests, noisy_image):
    img = np.ascontiguousarray(np.asarray(noisy_image, dtype=np.float32)[0])
    ests = np.asarray(ests, dtype=np.float32).reshape(HP * WP, 5)
    grid = np.linspace(-1.0, 1.0, R, dtype=np.float32)
    xg = np.tile(grid, R)
    yg = np.repeat(grid, R)
    in_maps = []
    for m in range(NCORES):
        in_maps.append({
            "img": np.ascontiguousarray(img[16 * m:16 * m + ROWS_PER_CORE]).reshape(-1),
            "ests": np.ascontiguousarray(
                ests[m * 512:(m + 1) * 512].reshape(GROUPS, 128, 5).transpose(1, 2, 0)).reshape(-1),
            "xg": xg, "yg": yg,
        })
    return in_maps


def assemble(results):
    out = np.empty((1, 3, 3, HP, WP), np.float32)
    for m in range(NCORES):
        r = results[m]["out"].reshape(2, 64, 3, 3, GROUPS)   # (dh, wp, c, k, g)
        out[0, :, :, 8 * m:8 * m + 8, :] = (
            r.transpose(2, 3, 4, 0, 1).reshape(3, 3, 8, WP))
    return out


def kernel(ests, noisy_image, gt_image=None, alpha=None, **_):
    nc = _get_nc()
    in_maps = make_in_maps(ests, noisy_image)
    res = run_bass_kernel_spmd(nc, in_maps, core_ids=list(range(NCORES)))
    return assemble(res.results)
